# revision 7
# baseline (speedup 1.0000x reference)
"""Trainium2 distributed kernel for the FM/GNN rating model (nn_APM_16733192585590).

Math: rate = x@w_lin + 0.5*sum((xV)^2 - x^2 V^2) + bias_u[uid] + bias_i[iid] + 3
with x = [user_table[uid], word_table[uid], item_table[iid], word_table[iid+NU]].

Decomposition (x = [u | i], V = [V_U ; V_I], G = V_U @ V_I.T):
  rate_b = alpha_u[uid_b] + alpha_i[iid_b] + U_emb[uid_b] @ G @ I_emb[iid_b]
where alpha_* fold the row-local linear + quadratic + bias terms (+1.5 each).
G is compressed to rank 126 (SVD; truncation error ~1e-8 rel) so each table
row is exactly 128 floats = 512B: [emb' (126) | alpha | sentinel].

Distribution: the batch is sharded by uid // 12500 (data-parallel with the
u-table row-sharded across the 8 cores), so each core's u-row gathers hit one
<=32k-row window and are served by a single GPSIMD dma_gather (SWDGE cost is
~1us fixed per instruction + 0.34ns/descriptor, so few big gathers beat many
small ones). Within a core, elements are sorted by iid // 25000 into 4
segments of whole 128-row columns, so the i-rows also need only 4 windowed
dma_gathers. Slot s maps to (partition s%128, column s//128) in both the
u-grid and i-grid; DVE multiplies the grids elementwise and reduces each row
to rate + host adds nothing (alphas ride along as sentinel products). The
host only permutes indices/outputs (weight prep is batch-independent).
"""

import numpy as np

from concourse import bacc, bass, mybir
from concourse import library_config
import concourse.tile as tile
from concourse.bass_utils import run_bass_kernel_spmd

N_USERS = 100000
N_ITEMS = 100000
DIM = 64
EMB = 2 * DIM          # 128 combined embedding floats per id
E = 128                # gathered row length (512B f32)
RK = 126               # rank kept of G; cols 126,127 hold [alpha|1]/[1|alpha]
BATCH = 16384
N_CORES = 8
P = 128
USHARD = N_USERS // N_CORES     # 12500 u-table rows per core
NWIN = 4
IWIN = 25000                    # i-table window (<=32767 for int16 idx)
SEG_COLS = 5                    # columns per i-window segment
SEG_CAP = SEG_COLS * P          # 640 slots per segment
COLS = NWIN * SEG_COLS          # 20 columns
CAP = COLS * P                  # 2560 slots per core

_nc_cache = {}


def _build_nc():
    if "nc" in _nc_cache:
        return _nc_cache["nc"]
    f32 = mybir.dt.float32
    i16 = mybir.dt.int16

    nc = bacc.Bacc(None, target_bir_lowering=False, debug=False)
    # idx tile: [u (CAP/16) | 4 x i-window (SEG_CAP/16)] int16 columns
    UC = CAP // 16
    SC = SEG_CAP // 16
    cidx = nc.declare_dram_parameter("cidx", [P, UC + NWIN * SC], i16, isOutput=False)
    utab = nc.declare_dram_parameter("utab", [USHARD, E], f32, isOutput=False)
    itab = nc.declare_dram_parameter("itab", [N_ITEMS, E], f32, isOutput=False)
    out = nc.declare_dram_parameter("out", [P, COLS], f32, isOutput=True)

    with tile.TileContext(nc) as tc:
        with tc.tile_pool(name="p", bufs=1) as pool:
            ci = pool.tile([P, UC + NWIN * SC], i16)
            xau = pool.tile([P, COLS, E], f32)
            xai = pool.tile([P, COLS, E], f32)
            nc.gpsimd.load_library(library_config.mlp)
            nc.sync.dma_start(ci[:], cidx[:])
            # SWDGE ring fits ~128 descs/engine-ring per instruction
            # (n/16+1 must stay under it), so split the u-gather into <=1024s
            for g, (s0, n) in enumerate([(0, 1024), (1024, 1024), (2048, 512)]):
                nc.gpsimd.dma_gather(
                    xau[:, s0 // P : (s0 + n) // P, :],
                    utab[:],
                    ci[:, s0 // 16 : (s0 + n) // 16],
                    n,
                    n,
                    E,
                )
            for w in range(NWIN):
                nc.gpsimd.dma_gather(
                    xai[:, w * SEG_COLS : (w + 1) * SEG_COLS, :],
                    itab[w * IWIN : (w + 1) * IWIN],
                    ci[:, UC + w * SC : UC + (w + 1) * SC],
                    SEG_CAP,
                    SEG_CAP,
                    E,
                )
            prod = pool.tile([P, COLS, E], f32)
            r = pool.tile([P, COLS], f32)
            nc.vector.tensor_tensor(
                out=prod[:], in0=xau[:], in1=xai[:], op=mybir.AluOpType.mult
            )
            nc.vector.reduce_sum(r[:], prod[:], axis=mybir.AxisListType.X)
            nc.sync.dma_start(out[:], r[:])

    nc.finalize()
    _nc_cache["nc"] = nc
    return nc


def _wrap_idxs(lst):
    """[n] -> [128, n//16] int16: element j at [j%16, j//16], replicated x8."""
    lst = np.asarray(lst, np.int16)
    a = lst.reshape(-1, 16).T
    return np.tile(a, (8, 1))


def _prep_tables(user_table, item_table, word_table, w_lin, V, bias_u, bias_i):
    """Weight-only preprocessing (batch-independent, reusable)."""
    f32 = np.float32
    U_emb = np.concatenate([user_table, word_table[:N_USERS]], axis=1).astype(f32)
    I_emb = np.concatenate(
        [item_table, word_table[N_USERS : N_USERS + N_ITEMS]], axis=1
    ).astype(f32)
    V = np.asarray(V, f32)
    w_lin = np.asarray(w_lin, f32)
    V_U, V_I = V[:EMB], V[EMB:]
    s = (V * V).sum(axis=1)
    ZU = U_emb @ V_U
    alpha_u = (
        np.asarray(bias_u, f32)
        + U_emb @ w_lin[:EMB]
        + 0.5 * (ZU * ZU).sum(axis=1)
        - 0.5 * (U_emb * U_emb) @ s[:EMB]
        + 1.5
    )
    ZI = I_emb @ V_I
    alpha_i = (
        np.asarray(bias_i, f32)
        + I_emb @ w_lin[EMB:]
        + 0.5 * (ZI * ZI).sum(axis=1)
        - 0.5 * (I_emb * I_emb) @ s[EMB:]
        + 1.5
    )
    G = V_U @ V_I.T
    Uu, sv, Vt = np.linalg.svd(G)
    A = (Uu[:, :RK] * sv[:RK]).astype(f32)
    B = Vt[:RK].T.astype(f32)
    utab = np.empty((N_USERS, E), f32)
    utab[:, :RK] = U_emb @ A
    utab[:, RK] = alpha_u
    utab[:, RK + 1] = 1.0
    itab = np.empty((N_ITEMS, E), f32)
    itab[:, :RK] = I_emb @ B
    itab[:, RK] = 1.0
    itab[:, RK + 1] = alpha_i
    return np.ascontiguousarray(utab), np.ascontiguousarray(itab)


def kernel(
    uid_batch,
    iid_batch,
    n_users,
    user_table,
    item_table,
    word_table,
    w_lin,
    V,
    bias_u,
    bias_i,
    _trace=False,
):
    uid = np.asarray(uid_batch).astype(np.int64)
    iid = np.asarray(iid_batch).astype(np.int64)
    utab, itab = _prep_tables(
        np.asarray(user_table, np.float32),
        np.asarray(item_table, np.float32),
        np.asarray(word_table, np.float32),
        w_lin,
        V,
        bias_u,
        bias_i,
    )

    core_of = uid // USHARD
    in_maps = []
    slot_to_batch = []
    for c in range(N_CORES):
        sel = np.where(core_of == c)[0]
        iw = iid[sel] // IWIN
        u16 = np.zeros(CAP, np.int16)
        i16 = np.zeros((NWIN, SEG_CAP), np.int16)
        s2b = np.full(CAP, -1, np.int64)
        for w in range(NWIN):
            ss = sel[iw == w]
            n = len(ss)
            if n > SEG_CAP:
                raise RuntimeError(
                    f"segment overflow core {c} window {w}: {n} > {SEG_CAP}"
                )
            base = w * SEG_CAP
            s2b[base : base + n] = ss
            u16[base : base + n] = uid[ss] - USHARD * c
            i16[w, :n] = iid[ss] - IWIN * w
        ci = np.concatenate(
            [_wrap_idxs(u16)] + [_wrap_idxs(i16[w]) for w in range(NWIN)], axis=1
        )
        slot_to_batch.append(s2b)
        in_maps.append(
            {
                "cidx": np.ascontiguousarray(ci),
                "utab": np.ascontiguousarray(utab[c * USHARD : (c + 1) * USHARD]),
                "itab": itab,
            }
        )

    nc = _build_nc()
    res = run_bass_kernel_spmd(
        nc, in_maps, core_ids=list(range(N_CORES)), trace=_trace
    )
    full = np.empty(BATCH, np.float32)
    for c in range(N_CORES):
        flat = np.asarray(res.results[c]["out"], np.float32).T.reshape(-1)
        s2b = slot_to_batch[c]
        valid = s2b >= 0
        full[s2b[valid]] = flat[valid]
    if _trace:
        return full, res
    return full


# revision 8
# speedup vs baseline: 1.1597x; 1.1597x over previous
"""Trainium2 distributed kernel for the FM/GNN rating model (nn_APM_16733192585590).

Math: rate = x@w_lin + 0.5*sum((xV)^2 - x^2 V^2) + bias_u[uid] + bias_i[iid] + 3
with x = [user_table[uid], word_table[uid], item_table[iid], word_table[iid+NU]].

Decomposition (x = [u | i], V = [V_U ; V_I], G_ui = V_U @ V_I.T):
  rate_b = alpha_u[uid_b] + alpha_i[iid_b] + U_emb[uid_b] @ G_ui @ I_emb[iid_b]
where alpha_* fold the row-local linear + quadratic + bias terms (+1.5 each).
Precomputing Uhat = U_emb @ G_ui per table row (weight-only transform) turns the
device work into two row gathers and a row-wise dot.

Device (per core, batch shard of 2048, batch b -> (partition b//16, tile b%16)):
  - one combined table: rows [Uhat | alpha_u | 1] and [I_emb | 1 | alpha_i]
  - 32 indirect row gathers (544B rows), u/i interleaved per tile
  - chunked DVE: mult + segmented reduce over 130 cols (sentinel columns make
    the reduce include alpha_u + alpha_i), overlapping later gathers
  - one contiguous output DMA. No transposes, no matmuls, no collectives.

Perf notes from this session (traces on this firmware):
  - SWDGE descriptor generation on GpSimd runs at ~8ns/descriptor regardless
    of batching (INDIRECT1D ~1.1us/128 descs; DMAGatherAnt ~6us/~730 descs),
    so the 4096 row-descriptors/core cost ~35us serial on GpSimd either way.
  - indirect_dma_start consumes only ONE offset per partition (128 descs max
    per instruction); multi-column offset APs gather garbage.
  - dma_gather (mlp library) works but needs int16 indices (<=32k-row window),
    n/16+1 <= ~128 ring descs per instruction, and gave 71us vs 61us here.
"""

import numpy as np

from concourse import bacc, bass, mybir
import concourse.tile as tile
from concourse.bass_utils import run_bass_kernel_spmd

N_USERS = 100000
N_ITEMS = 100000
DIM = 64
EMB = 2 * DIM          # 128 combined embedding floats per row
RED = EMB + 2          # reduce range: emb dot + alpha_u + alpha_i sentinels
R = 136                # padded row length (544B)
BATCH = 16384
N_CORES = 8
SHARD = BATCH // N_CORES      # 2048
P = 128
T = SHARD // P                # 16 tiles of 128 batch elements
CHUNK = 2                     # tiles per DVE chunk (smaller tail)

_nc_cache = {}


def _build_nc(finalize=True):
    if finalize and "nc" in _nc_cache:
        return _nc_cache["nc"]
    f32 = mybir.dt.float32
    i32 = mybir.dt.int32

    nc = bacc.Bacc(None, target_bir_lowering=False, debug=False)
    cidx = nc.declare_dram_parameter("cidx", [P, 2 * T], i32, isOutput=False)
    ctab = nc.declare_dram_parameter("ctab", [N_USERS + N_ITEMS, R], f32, isOutput=False)
    out = nc.declare_dram_parameter("out", [P, T], f32, isOutput=True)

    with tile.TileContext(nc) as tc:
        with tc.tile_pool(name="p", bufs=1) as pool:
            ci = pool.tile([P, 2 * T], i32)
            nc.sync.dma_start(ci[:], cidx[:])
            xa = pool.tile([P, 2 * T, R], f32)
            prod = pool.tile([P, CHUNK, RED], f32)
            r = pool.tile([P, T], f32)
            for g in range(T // CHUNK):
                for t in range(g * CHUNK, (g + 1) * CHUNK):
                    for k in (2 * t, 2 * t + 1):  # u-col then i-col of tile t
                        nc.gpsimd.indirect_dma_start(
                            out=xa[:, k, :],
                            out_offset=None,
                            in_=ctab[:],
                            in_offset=bass.IndirectOffsetOnAxis(
                                ap=ci[:, k : k + 1], axis=0
                            ),
                        )
                c0 = 2 * g * CHUNK
                nc.vector.tensor_tensor(
                    out=prod[:],
                    in0=xa[:, c0 : c0 + 2 * CHUNK : 2, 0:RED],
                    in1=xa[:, c0 + 1 : c0 + 2 * CHUNK : 2, 0:RED],
                    op=mybir.AluOpType.mult,
                )
                nc.vector.reduce_sum(
                    r[:, g * CHUNK : (g + 1) * CHUNK],
                    prod[:],
                    axis=mybir.AxisListType.X,
                )
            nc.sync.dma_start(out[:], r[:])

    if finalize:
        nc.finalize()
        _nc_cache["nc"] = nc
    else:
        nc.compile()
    return nc


def _prep_tables(user_table, item_table, word_table, w_lin, V, bias_u, bias_i):
    """Weight-only preprocessing (reusable across batches)."""
    f32 = np.float32
    U_emb = np.concatenate([user_table, word_table[:N_USERS]], axis=1).astype(f32)
    I_emb = np.concatenate(
        [item_table, word_table[N_USERS : N_USERS + N_ITEMS]], axis=1
    ).astype(f32)
    V = np.asarray(V, f32)
    w_lin = np.asarray(w_lin, f32)
    V_U, V_I = V[:EMB], V[EMB:]
    s = (V * V).sum(axis=1)
    ZU = U_emb @ V_U
    alpha_u = (
        np.asarray(bias_u, f32)
        + U_emb @ w_lin[:EMB]
        + 0.5 * (ZU * ZU).sum(axis=1)
        - 0.5 * (U_emb * U_emb) @ s[:EMB]
        + 1.5
    )
    ZI = I_emb @ V_I
    alpha_i = (
        np.asarray(bias_i, f32)
        + I_emb @ w_lin[EMB:]
        + 0.5 * (ZI * ZI).sum(axis=1)
        - 0.5 * (I_emb * I_emb) @ s[EMB:]
        + 1.5
    )
    Uhat = U_emb @ (V_U @ V_I.T)
    ctab = np.zeros((N_USERS + N_ITEMS, R), f32)
    ctab[:N_USERS, :EMB] = Uhat
    ctab[:N_USERS, EMB] = alpha_u
    ctab[:N_USERS, EMB + 1] = 1.0
    ctab[N_USERS:, :EMB] = I_emb
    ctab[N_USERS:, EMB] = 1.0
    ctab[N_USERS:, EMB + 1] = alpha_i
    return np.ascontiguousarray(ctab)


def kernel(
    uid_batch,
    iid_batch,
    n_users,
    user_table,
    item_table,
    word_table,
    w_lin,
    V,
    bias_u,
    bias_i,
    _trace=False,
):
    uid = np.asarray(uid_batch).astype(np.int32)
    iid = np.asarray(iid_batch).astype(np.int32) + N_USERS
    ctab = _prep_tables(
        np.asarray(user_table, np.float32),
        np.asarray(item_table, np.float32),
        np.asarray(word_table, np.float32),
        w_lin,
        V,
        bias_u,
        bias_i,
    )

    nc = _build_nc()
    in_maps = []
    for c in range(N_CORES):
        us = uid[c * SHARD : (c + 1) * SHARD].reshape(P, T)
        is_ = iid[c * SHARD : (c + 1) * SHARD].reshape(P, T)
        cidx = np.empty((P, 2 * T), np.int32)
        cidx[:, 0::2] = us
        cidx[:, 1::2] = is_
        in_maps.append({"cidx": np.ascontiguousarray(cidx), "ctab": ctab})
    res = run_bass_kernel_spmd(
        nc, in_maps, core_ids=list(range(N_CORES)), trace=_trace
    )
    outs = [res.results[c]["out"].reshape(SHARD) for c in range(N_CORES)]
    full = np.concatenate(outs).astype(np.float32)
    if _trace:
        return full, res
    return full
